# revision 13
# baseline (speedup 1.0000x reference)
"""v4: ACT row-major + DVE transposed with PE (tensor engine) reduction.

ACT keeps CA cols/row row-major with accum_out (free reduction on ScalarE).
DVE share (CD = 155 blocks of 128 classes) moves to a transposed layout:
xd[p, b*512 + rr] = x[row rr, CA + b*128 + p]. One tensor_scalar per chunk
does the Schraudolph bit-trick exp (fp8 -> i16, 2x mode); the Tensor
engine then reduces each 128-class block with a ones-matmul into a
single PSUM bank (psD[1, 512] accumulates all 155 blocks, f32). This
drops the second DVE pass entirely: DVE ~42us, ACT ~43us, PE ~34us,
DMA ~40us, all overlapped.
"""

import sys

import numpy as np

sys.path.insert(0, "/opt/trn_rl_repo")

BATCH = 4096
C = 32000
NCORES = 8
P = 128
ROWS = BATCH // NCORES  # 512
RPP = ROWS // P  # 4
CA = 12160  # ACT columns per row
CD = C - CA  # 19840 = 155 blocks of 128
NBLK = CD // P  # 155
FA = RPP * CA  # 48640 bytes/line (fp8)
FD = NBLK * ROWS  # 79360 bytes/line (fp8)
GS = [8, 16, 24, 24, 24, 24, 24, 8, 3]  # blocks per DVE chunk (small ramp + tiny tail)
assert sum(GS) == NBLK
# ACT windows (row, col_off, width): row 0 split so ACT starts ~5us earlier
A_WIN = [(0, 0, 2000), (0, 2000, CA - 2000), (1, 0, CA), (2, 0, CA), (3, 0, CA)]
NSLOT = len(A_WIN)
# DMA/compute emission order: ("d", chunk_idx) or ("a", win_idx)
SCHED = [("d", 0), ("a", 0), ("a", 1), ("d", 1), ("d", 2), ("a", 2),
         ("d", 3), ("a", 3), ("d", 4), ("a", 4), ("d", 5), ("d", 6),
         ("d", 7), ("d", 8)]

A16 = float(128.0 / np.log(2.0))
B16 = float(127 * 128 - 0.058 * 128)

_CACHE: dict = {}


def _build_nc():
    import concourse.bacc as bacc
    import concourse.tile as tile
    from concourse import mybir

    nc = bacc.Bacc(
        "TRN2", target_bir_lowering=False, debug=False, num_devices=NCORES
    )
    xa = nc.dram_tensor("xa", [P, FA], mybir.dt.float8e4, kind="ExternalInput")
    xd = nc.dram_tensor("xd", [P, FD], mybir.dt.float8e4, kind="ExternalInput")
    sums_out = nc.dram_tensor(
        "sums", [P, NSLOT], mybir.dt.float32, kind="ExternalOutput"
    )
    sd_out = nc.dram_tensor("sd", [1, ROWS], mybir.dt.float32, kind="ExternalOutput")

    with tile.TileContext(nc) as tc:
        with (
            tc.tile_pool(name="xa", bufs=4) as xapool,
            tc.tile_pool(name="xd", bufs=6) as xdpool,
            tc.tile_pool(name="ea", bufs=1) as eapool,
            tc.tile_pool(name="it", bufs=2) as itpool,
            tc.tile_pool(name="s", bufs=1) as spool,
            tc.tile_pool(name="ps", bufs=1, space="PSUM") as pspool,
        ):
            sums = spool.tile([P, NSLOT], mybir.dt.float32, tag="sums")
            sd_sb = spool.tile([1, ROWS], mybir.dt.float32, tag="sd_sb")
            ones = spool.tile([P, 1], mybir.dt.bfloat16, tag="ones")
            nc.vector.memset(ones[:, :], 1.0)
            psD = pspool.tile([1, ROWS], mybir.dt.float32, tag="psD")

            blk = 0
            for kind, idx in SCHED:
                if kind == "d":
                    g = GS[idx]
                    xd_t = xdpool.tile([P, g * ROWS], mybir.dt.float8e4, tag="xd")
                    off = sum(GS[:idx]) * ROWS
                    nc.sync.dma_start(out=xd_t[:, :], in_=xd[:, off : off + g * ROWS])
                    it_t = itpool.tile([P, g * ROWS], mybir.dt.int16, tag="it")
                    nc.vector.tensor_scalar(
                        it_t[:, :], xd_t[:, :], A16, B16,
                        mybir.AluOpType.mult, mybir.AluOpType.add,
                    )
                    it_bf = it_t[:, :].bitcast(mybir.dt.bfloat16)
                    for k in range(g):
                        nc.tensor.matmul(
                            psD[0:1, :],
                            ones[:, 0:1],
                            it_bf[:, k * ROWS : (k + 1) * ROWS],
                            start=(blk == 0),
                            stop=(blk == NBLK - 1),
                        )
                        blk += 1
                else:
                    r, co, w = A_WIN[idx]
                    xa_t = xapool.tile([P, w], mybir.dt.float8e4, tag="xa")
                    nc.sync.dma_start(
                        out=xa_t[:, :], in_=xa[:, r * CA + co : r * CA + co + w]
                    )
                    # write-only scratch; fp8e5 halves SBUF write traffic and
                    # footprint (accum_out is computed at f32 internally);
                    # e5m2 range covers exp([-6, 6]) with no overflow
                    ea_t = eapool.tile([P, w], mybir.dt.float8e5, tag="ea")
                    nc.scalar.activation(
                        out=ea_t[:, :],
                        in_=xa_t[:, :],
                        func=mybir.ActivationFunctionType.Exp,
                        accum_out=sums[:, idx : idx + 1],
                    )
            nc.scalar.copy(sd_sb[0:1, :], psD[0:1, :])
            nc.sync.dma_start(out=sums_out[:, :], in_=sums[:])
            nc.sync.dma_start(out=sd_out[0:1, :], in_=sd_sb[0:1, :])
    nc.compile()
    return nc


def get_nc():
    if "nc" not in _CACHE:
        _CACHE["nc"] = _build_nc()
    return _CACHE["nc"]


def make_in_maps(predicts: np.ndarray, targets: np.ndarray) -> list[dict]:
    import ml_dtypes

    x8 = np.ascontiguousarray(predicts, dtype=np.float32).astype(
        ml_dtypes.float8_e4m3
    )
    in_maps = []
    for cix in range(NCORES):
        xc = x8[cix * ROWS : (cix + 1) * ROWS]  # [512, 32000], row rr = p*4+r
        xa = np.ascontiguousarray(xc[:, :CA].reshape(P, FA))
        # xd[p, b*512 + rr] = xc[rr, CA + b*128 + p]
        xd = np.ascontiguousarray(
            xc[:, CA:].reshape(ROWS, NBLK, P).transpose(2, 1, 0).reshape(P, FD)
        )
        in_maps.append({"xa": xa, "xd": xd})
    return in_maps


def kernel(predicts: np.ndarray, targets: np.ndarray) -> np.ndarray:
    from concourse.bass_utils import run_bass_kernel_spmd

    nc = get_nc()
    predicts = np.ascontiguousarray(predicts, dtype=np.float32)
    targets = np.asarray(targets).astype(np.int64)
    in_maps = make_in_maps(predicts, targets)
    res = run_bass_kernel_spmd(nc, in_maps, list(range(NCORES)))

    lse_total = np.float64(0.0)
    for cix in range(NCORES):
        s = np.asarray(res.results[cix]["sums"], dtype=np.float64)  # [P, NSLOT]
        sa = np.zeros((P, RPP))
        for idx, (r, co, w) in enumerate(A_WIN):
            sa[:, r] += s[:, idx]
        sd = np.asarray(res.results[cix]["sd"], dtype=np.float64)  # [1, ROWS]
        rowsum = sa.reshape(ROWS) + sd.reshape(ROWS)  # row rr = p*4+r order
        lse_total += np.log(rowsum).sum()
    picked = predicts[np.arange(BATCH), targets].astype(np.float64)
    loss = (lse_total - picked.sum()) / BATCH
    return np.asarray(loss, dtype=np.float32)
